# revision 12
# baseline (speedup 1.0000x reference)
"""Trainium2 Bass kernel for nn_MultiHeadAttention_89670327206616.

MultiHead attention with distance-decay (gamma) rescoring:
  qh,kh,vh = per-head projections of q,k,v
  s = qh@kh^T/sqrt(dk); p = softmax(causal_mask(s))
  R[q,j] = sum_{j'>j} p[q,j']          (suffix mass, via cumsum)
  dist = sqrt(R * |q-j|); te = exp(-softplus(gamma)*dist)  (clip is a no-op)
  attn = softmax(causal_mask(s*te)); out = (attn@vh) heads-concat @ Wo.T + bo

Sharding: data-parallel over batch, 1 of the 8 batch elements per NeuronCore.
Each core runs an identical single-core program; no collectives.

Key device-side design notes:
  - ACT (ScalarE) runs ONLY Exp/Ln -> a single activation table set
    (natural_log_exp_and_others), no ACT_TABLE_LOAD thrash. All PSUM
    evacuations are DVE copies / tensor_scalar ops.
    sqrt is computed as exp(0.5*ln(x)) for the same reason.
  - cumsum along keys via DVE tensor_tensor_scan; suffix R = denom - cumsum is
    computed inside the Ln activation (scale=-1, bias=denom*(1+2^-22)). denom
    is the scan's last column so (denom - cumsum) >= 0 exactly.
  - causal mask enters as a [-1e30 strict-upper] constant accumulated into the
    diagonal score block by one extra identity matmul (PSUM accumulate).
  - |q-j| enters in log domain: host passes ln(|q-j|) strips (0 -> -1e4).
  - softmax2: denom2 comes free from the exp's accum_out; attn is normalized
    (and cast to bf16) before the per-block PE transposes; attn^T strips are
    kept per key-block so attn@V runs as few wide-N bf16 matmuls with the
    vh tile stationary. attn/vh in bf16 (~0.3% component error), everything
    else fp32.
"""

import sys

for _p in ("/opt/trn_rl_repo",):
    if _p not in sys.path:
        sys.path.insert(0, _p)

import numpy as np

B, S, DM, H, DK = 8, 1024, 512, 8, 64
NCORES = 8
QB = S // 128  # 8 query blocks of 128

_CACHE = {}


def _build_program():
    import concourse.bass as bass
    import concourse.tile as tile
    from concourse import bacc, mybir

    # Pin ACT to the one table set containing Exp AND Ln, so the table-load
    # placement pass can never alternate sets (each reload costs ~2.7us and
    # this kernel interleaves Exp/Ln tightly). Ids must stay aligned with
    # act_info.json, so keep every set name but empty the others.
    if not getattr(bacc, "_act_tables_pinned", False):
        _orig_gat = bacc.get_activation_tables

        def _pinned_tables(arch):
            t = _orig_gat(arch)
            keep = "natural_log_exp_and_others"
            return {k: (v if k == keep else set()) for k, v in t.items()}

        bacc.get_activation_tables = _pinned_tables
        bacc._act_tables_pinned = True

    f32 = mybir.dt.float32
    bf16 = mybir.dt.bfloat16
    AF = mybir.ActivationFunctionType
    ALU = mybir.AluOpType

    nc = bacc.Bacc(None)

    # ---- DRAM parameters (per core) ----
    qT_d = nc.declare_dram_parameter("qT", [DM, S], f32, isOutput=False)
    kT_d = nc.declare_dram_parameter("kT", [DM, S], f32, isOutput=False)
    vT_d = nc.declare_dram_parameter("vT", [DM, S], f32, isOutput=False)
    WqT_d = nc.declare_dram_parameter("WqT", [DM, DM], f32, isOutput=False)
    WkT_d = nc.declare_dram_parameter("WkT", [DM, DM], f32, isOutput=False)
    WvT_d = nc.declare_dram_parameter("WvT", [DM, DM], f32, isOutput=False)
    WoT_d = nc.declare_dram_parameter("WoT", [DM, DM], f32, isOutput=False)
    bq_d = nc.declare_dram_parameter("bq2d", [DM, 1], f32, isOutput=False)
    bk_d = nc.declare_dram_parameter("bk2d", [DM, 1], f32, isOutput=False)
    bv_d = nc.declare_dram_parameter("bvrow", [1, DM], f32, isOutput=False)
    bo_d = nc.declare_dram_parameter("bo2d", [DM, 1], f32, isOutput=False)
    lngam_d = nc.declare_dram_parameter("lngam_bc", [128, H], f32, isOutput=False)
    lnpos_d = nc.declare_dram_parameter("lnpos", [QB, 128, S], f32, isOutput=False)
    I128_d = nc.declare_dram_parameter("I128", [128, 128], f32, isOutput=False)
    yT_d = nc.declare_dram_parameter("yT", [DM, S], f32, isOutput=True)

    NDMC = DM // 128  # 4 chunks of the model dim

    with tile.TileContext(nc) as tc:
        import contextlib

        persist = tc.tile_pool(name="persist", bufs=1)
        with contextlib.ExitStack() as stack:
            pp = stack.enter_context(persist)

            qhT = [pp.tile([128, S], f32, tag=f"qhT{i}", name=f"qhT{i}") for i in range(NDMC)]
            khT = [pp.tile([128, S], f32, tag=f"khT{i}", name=f"khT{i}") for i in range(NDMC)]
            vh = [pp.tile([128, DM], bf16, tag=f"vh{i}", name=f"vh{i}") for i in range(QB)]
            ccT = [pp.tile([128, S], f32, tag=f"ccT{t}", name=f"ccTs{t}") for t in range(NDMC)]
            I128 = pp.tile([128, 128], f32, tag="I128", name="I128")
            I128b = pp.tile([128, 128], bf16, tag="I128b", name="I128b")
            lngam = pp.tile([128, H], f32, tag="lngam", name="lngam")
            bq = [pp.tile([128, 1], f32, tag=f"bq{t}", name=f"bq{t}") for t in range(NDMC)]
            bk = [pp.tile([128, 1], f32, tag=f"bk{t}", name=f"bk{t}") for t in range(NDMC)]
            bo = [pp.tile([128, 1], f32, tag=f"bo{t}", name=f"bo{t}") for t in range(NDMC)]

            bump = pp.tile([128, 1], f32, tag="bump", name="bump")
            nc.vector.memset(bump, 1.0 + 2.0 ** -22)
            nc.sync.dma_start(out=I128, in_=I128_d[:, :])
            nc.sync.dma_start(out=lngam, in_=lngam_d[:, :])
            nc.vector.tensor_copy(I128b, I128)
            for t in range(NDMC):
                nc.sync.dma_start(out=bq[t], in_=bq_d[t * 128:(t + 1) * 128, :])
                nc.sync.dma_start(out=bk[t], in_=bk_d[t * 128:(t + 1) * 128, :])
                nc.sync.dma_start(out=bo[t], in_=bo_d[t * 128:(t + 1) * 128, :])

            # ---------- phase 1: projections ----------
            with tc.tile_pool(name="p1in", bufs=1) as p1in, \
                 tc.tile_pool(name="p1ps", bufs=3, space="PSUM") as p1ps:
                qTt = [p1in.tile([128, S], f32, tag=f"qT{t}", name=f"qTs{t}") for t in range(NDMC)]
                kTt = [p1in.tile([128, S], f32, tag=f"kT{t}", name=f"kTs{t}") for t in range(NDMC)]
                vTt = [p1in.tile([128, S], f32, tag=f"vT{t}", name=f"vTs{t}") for t in range(NDMC)]
                Wq = [p1in.tile([128, DM], f32, tag=f"Wq{t}", name=f"Wqs{t}") for t in range(NDMC)]
                Wk = [p1in.tile([128, DM], f32, tag=f"Wk{t}", name=f"Wks{t}") for t in range(NDMC)]
                Wv = [p1in.tile([128, DM], f32, tag=f"Wv{t}", name=f"Wvs{t}") for t in range(NDMC)]
                ones1 = p1in.tile([1, 128], f32, tag="ones1", name="ones1")
                bvr = p1in.tile([1, DM], f32, tag="bvr", name="bvr")
                nc.vector.memset(ones1, 1.0)
                nc.sync.dma_start(out=bvr, in_=bv_d[:, :])
                for t in range(NDMC):
                    sl = slice(t * 128, (t + 1) * 128)
                    nc.sync.dma_start(out=qTt[t], in_=qT_d[sl, :])
                    nc.sync.dma_start(out=kTt[t], in_=kT_d[sl, :])
                    nc.sync.dma_start(out=vTt[t], in_=vT_d[sl, :])
                    nc.sync.dma_start(out=Wq[t], in_=WqT_d[sl, :])
                    nc.sync.dma_start(out=Wk[t], in_=WkT_d[sl, :])
                    nc.sync.dma_start(out=Wv[t], in_=WvT_d[sl, :])

                # qhT/khT: [d', s] = sum_dm WxT[dm, d'] * xT[dm, s]  (+ bias)
                for dst, W, xT, bias in ((qhT, Wq, qTt, bq), (khT, Wk, kTt, bk)):
                    for i in range(NDMC):           # d' block
                        for n in range(S // 512):   # s chunk
                            ps = p1ps.tile([128, 512], f32, tag="pp", name="pp")
                            for t in range(NDMC):   # dm chunk
                                nc.tensor.matmul(
                                    ps,
                                    lhsT=W[t][:, i * 128:(i + 1) * 128],
                                    rhs=xT[t][:, n * 512:(n + 1) * 512],
                                    start=(t == 0), stop=(t == NDMC - 1),
                                )
                            nc.vector.tensor_scalar_add(
                                dst[i][:, n * 512:(n + 1) * 512], ps, bias[i],
                            )
                # vh: [s, d'] = sum_dm vT[dm, s-block]^T * WvT[dm, d'] (+ bv)
                for sb in range(QB):
                    ps = p1ps.tile([128, DM], f32, tag="pp", name="pp")
                    for t in range(NDMC):
                        nc.tensor.matmul(
                            ps,
                            lhsT=vTt[t][:, sb * 128:(sb + 1) * 128],
                            rhs=Wv[t][:, :],
                            start=(t == 0), stop=False,
                        )
                    nc.tensor.matmul(ps, lhsT=ones1, rhs=bvr, start=False, stop=True)
                    nc.vector.tensor_copy(vh[sb], ps)  # fp32 psum -> bf16

            # ---------- phase 2: attention (h outer, qb inner) ----------
            with tc.tile_pool(name="p2w", bufs=4) as p2w, \
                 tc.tile_pool(name="p2t", bufs=8) as p2t, \
                 tc.tile_pool(name="lnp", bufs=3) as lnp, \
                 tc.tile_pool(name="att", bufs=2) as attp, \
                 tc.tile_pool(name="ps_s", bufs=3, space="PSUM") as ps_s, \
                 tc.tile_pool(name="ps_o", bufs=1, space="PSUM") as ps_o:
                for h in range(H):
                    hi, hr = h // 2, (h % 2) * 64
                    # attn^T strips for this head: block (kb, qb) lives at
                    # [:, kb*S + qb*128 : kb*S + (qb+1)*128]
                    attnT = attp.tile([128, QB * S], bf16, tag="attnT", name="attnT")
                    attnT3 = attnT.rearrange("p (kb q) -> p kb q", kb=QB)
                    for qb in range(QB):
                        W = (qb + 1) * 128
                        lnpos = lnp.tile([128, S], f32, tag="lnpos", name="lnpos")
                        nc.gpsimd.dma_start(out=lnpos[:, :W], in_=lnpos_d[qb, :, :W])
                        qa = qhT[hi][hr:hr + 64, qb * 128:(qb + 1) * 128]
                        pss = ps_s.tile([128, 1024], f32, tag="pss", name="pss")
                        bounds = [(c, min(c + 512, W)) for c in range(0, W, 512)]
                        for (c0, c1) in bounds:
                            nc.tensor.matmul(
                                pss[:, c0:c1],
                                lhsT=qa, rhs=khT[hi][hr:hr + 64, c0:c1],
                                start=True, stop=True,
                            )
                        sraw = p2w.tile([128, S], f32, tag="sraw", name="sraw")
                        nc.vector.tensor_copy(sraw[:, :W], pss[:, :W])
                        # causal mask on the diagonal block: (q - j) >= 0 keeps
                        nc.gpsimd.affine_select(
                            out=sraw[:, qb * 128:W], in_=sraw[:, qb * 128:W],
                            compare_op=ALU.is_ge, fill=-1e30,
                            base=0, pattern=[[-1, 128]], channel_multiplier=1,
                        )
                        u = p2w.tile([128, S], f32, tag="u", name="u")
                        nc.scalar.activation(u[:, :W], sraw[:, :W], AF.Exp, scale=0.125)
                        cs = p2w.tile([128, S], f32, tag="cs", name="cs")
                        nc.vector.tensor_tensor_scan(
                            cs[:, :W], data0=u[:, :W], data1=u[:, :W],
                            initial=0.0, op0=ALU.add, op1=ALU.bypass,
                        )
                        nrden = p2t.tile([128, 1], f32, tag="nrden", name="nrden")
                        nc.vector.reciprocal(nrden, cs[:, W - 1:W])
                        nc.vector.tensor_scalar_mul(nrden, nrden, -1.0)
                        # lnr = ln(R/denom + ~2^-22); suffix mass ratio in log domain
                        lnr = p2w.tile([128, S], f32, tag="lnr", name="lnr")
                        nc.scalar.activation(
                            lnr[:, :W], cs[:, :W], AF.Ln, scale=nrden,
                            bias=bump,
                        )
                        # lnsum = lnr + lnpos, in place (frees a tile slot)
                        nc.gpsimd.tensor_tensor(
                            lnr[:, :W], lnr[:, :W], lnpos[:, :W], op=ALU.add,
                        )
                        tg = p2w.tile([128, S], f32, tag="tg", name="tg")
                        nc.scalar.activation(
                            tg[:, :W], lnr[:, :W], AF.Exp, scale=0.5,
                            bias=lngam[:, h:h + 1],
                        )
                        nc.scalar.activation(tg[:, :W], tg[:, :W], AF.Exp, scale=-1.0)
                        nc.vector.tensor_tensor(
                            tg[:, :W], sraw[:, :W], tg[:, :W], op=ALU.mult,
                        )
                        denom2 = p2t.tile([128, 1], f32, tag="den2", name="den2")
                        nc.scalar.activation(
                            tg[:, :W], tg[:, :W], AF.Exp, scale=0.125,
                            accum_out=denom2,
                        )
                        rden2 = p2t.tile([128, 1], f32, tag="rden2", name="rden2")
                        nc.vector.reciprocal(rden2, denom2)
                        attn = p2w.tile([128, S], bf16, tag="attn", name="attn")
                        nc.vector.tensor_scalar_mul(attn[:, :W], tg[:, :W], rden2)
                        # transpose attn blocks into per-kb strips via DMA xbar
                        for kb in range(qb + 1):
                            c0 = kb * 128
                            nc.sync.dma_start_transpose(
                                out=attnT3[:, kb, qb * 128:(qb + 1) * 128],
                                in_=attn[:, c0:c0 + 128],
                            )
                    # attn@V for this head: out^T[d, q] += vh_kb^T @ attnT_kb
                    pso = ps_o.tile([64, S], f32, tag="pso", name="pso")
                    for bank in range(2):           # psum bank = 512 q columns
                        q0 = bank * 512
                        kbs = [kb for kb in range(QB) if kb * 128 < q0 + 512]
                        for n, kb in enumerate(kbs):
                            c0 = max(kb * 128, q0)
                            nc.tensor.matmul(
                                pso[:, c0:q0 + 512],
                                lhsT=vh[kb][:, h * DK:(h + 1) * DK],
                                rhs=attnT3[:, kb, c0:q0 + 512],
                                start=(n == 0), stop=(n == len(kbs) - 1),
                            )
                    nc.vector.tensor_copy(ccT[hi][hr:hr + 64, :], pso)

            # ---------- phase 3: output projection ----------
            with tc.tile_pool(name="p3w", bufs=1) as p3w, \
                 tc.tile_pool(name="ps_y", bufs=2, space="PSUM") as ps_y:
                Wo = [p3w.tile([128, DM], f32, tag=f"Wo{t}", name=f"Wos{t}") for t in range(NDMC)]
                for t in range(NDMC):
                    nc.sync.dma_start(out=Wo[t], in_=WoT_d[t * 128:(t + 1) * 128, :])
                yTt = [p3w.tile([128, S], f32, tag=f"yT{m}", name=f"yTt{m}") for m in range(NDMC)]
                for mb in range(NDMC):
                    for n in range(S // 512):
                        psy = ps_y.tile([128, 512], f32, tag="psy", name="psy")
                        for t in range(NDMC):
                            nc.tensor.matmul(
                                psy,
                                lhsT=Wo[t][:, mb * 128:(mb + 1) * 128],
                                rhs=ccT[t][:, n * 512:(n + 1) * 512],
                                start=(t == 0), stop=(t == NDMC - 1),
                            )
                        nc.vector.tensor_scalar_add(
                            yTt[mb][:, n * 512:(n + 1) * 512], psy, bo[mb],
                        )
                for mb in range(NDMC):
                    nc.sync.dma_start(
                        out=yT_d[mb * 128:(mb + 1) * 128, :], in_=yTt[mb][:, :],
                    )

    nc.compile()
    return nc


def _get_program():
    if "nc" not in _CACHE:
        _CACHE["nc"] = _build_program()
    return _CACHE["nc"]


def _host_prep(inputs):
    """Build the per-core input maps from the full problem inputs."""
    f = np.float32
    q, k, v = inputs["q"], inputs["k"], inputs["v"]
    gammas = np.asarray(inputs["gammas"], dtype=f).reshape(H)

    common = {
        "WqT": np.ascontiguousarray(np.asarray(inputs["Wq"], f).T),
        "WkT": np.ascontiguousarray(np.asarray(inputs["Wk"], f).T),
        "WvT": np.ascontiguousarray(np.asarray(inputs["Wv"], f).T),
        "WoT": np.ascontiguousarray(np.asarray(inputs["Wo"], f).T),
        "bq2d": np.asarray(inputs["bq"], f).reshape(DM, 1).copy(),
        "bk2d": np.asarray(inputs["bk"], f).reshape(DM, 1).copy(),
        "bvrow": np.asarray(inputs["bv"], f).reshape(1, DM).copy(),
        "bo2d": np.asarray(inputs["bo"], f).reshape(DM, 1).copy(),
    }
    # ln(softplus(gamma_h)), broadcast to all 128 partitions
    lngam = np.log(np.logaddexp(0.0, gammas.astype(np.float64))).astype(f)
    common["lngam_bc"] = np.ascontiguousarray(np.broadcast_to(lngam, (128, H)))
    # ln|q-j| strips: strip[qb][i, j] = ln(qb*128 + i - j) for valid, else -1e4
    i_idx = np.arange(128)[:, None]
    j_idx = np.arange(S)[None, :]
    lnpos = np.full((QB, 128, S), -1e4, dtype=f)
    for qb in range(QB):
        pos = qb * 128 + i_idx - j_idx
        with np.errstate(divide="ignore", invalid="ignore"):
            lp = np.where(pos > 0, np.log(np.maximum(pos, 1)), -1e4)
        lnpos[qb] = lp.astype(f)
    common["lnpos"] = lnpos
    common["I128"] = np.eye(128, dtype=f)

    in_maps = []
    for c in range(NCORES):
        m = dict(common)
        m["qT"] = np.ascontiguousarray(np.asarray(q[c], f).T)
        m["kT"] = np.ascontiguousarray(np.asarray(k[c], f).T)
        m["vT"] = np.ascontiguousarray(np.asarray(v[c], f).T)
        in_maps.append(m)
    return in_maps


def run(inputs, trace=False):
    """Run on the 8 NeuronCores; returns (output [B,S,DM], BassKernelResults)."""
    from concourse.bass_utils import run_bass_kernel_spmd

    nc = _get_program()
    in_maps = _host_prep(inputs)
    res = run_bass_kernel_spmd(nc, in_maps, list(range(NCORES)), trace=trace)
    out = np.stack(
        [np.ascontiguousarray(res.results[c]["yT"].T) for c in range(NCORES)]
    ).astype(np.float32)
    return out, res


def kernel(**inputs) -> np.ndarray:
    out, _ = run(inputs)
    return out


# revision 13
# speedup vs baseline: 1.4055x; 1.4055x over previous
"""Trainium2 Bass kernel for nn_MultiHeadAttention_89670327206616.

MultiHead attention with distance-decay (gamma) rescoring:
  qh,kh,vh = per-head projections of q,k,v
  s = qh@kh^T/sqrt(dk); p = softmax(causal_mask(s))
  R[q,j] = sum_{j'>j} p[q,j']          (suffix mass, via cumsum)
  dist = sqrt(R * |q-j|); te = exp(-softplus(gamma)*dist)  (clip is a no-op)
  attn = softmax(causal_mask(s*te)); out = (attn@vh) heads-concat @ Wo.T + bo

Sharding: data-parallel over batch, 1 of the 8 batch elements per NeuronCore.
Each core runs an identical single-core program; no collectives.

Key device-side design notes:
  - ACT (ScalarE) runs ONLY Exp/Ln -> a single activation table set
    (natural_log_exp_and_others), no ACT_TABLE_LOAD thrash. All PSUM
    evacuations are DVE copies / tensor_scalar ops.
    sqrt is computed as exp(0.5*ln(x)) for the same reason.
  - cumsum along keys via DVE tensor_tensor_scan; suffix R = denom - cumsum is
    computed inside the Ln activation (scale=-1, bias=denom*(1+2^-22)). denom
    is the scan's last column so (denom - cumsum) >= 0 exactly.
  - causal mask enters as a [-1e30 strict-upper] constant accumulated into the
    diagonal score block by one extra identity matmul (PSUM accumulate).
  - |q-j| enters in log domain: host passes ln(|q-j|) strips (0 -> -1e4).
  - softmax2: denom2 comes free from the exp's accum_out; attn is normalized
    (and cast to bf16) before the per-block PE transposes; attn^T strips are
    kept per key-block so attn@V runs as few wide-N bf16 matmuls with the
    vh tile stationary. attn/vh in bf16 (~0.3% component error), everything
    else fp32.
"""

import sys

for _p in ("/opt/trn_rl_repo",):
    if _p not in sys.path:
        sys.path.insert(0, _p)

import numpy as np

B, S, DM, H, DK = 8, 1024, 512, 8, 64
NCORES = 8
QB = S // 128  # 8 query blocks of 128

_CACHE = {}


def _build_program():
    import concourse.bass as bass
    import concourse.tile as tile
    from concourse import bacc, mybir

    # Pin ACT to the one table set containing Exp AND Ln, so the table-load
    # placement pass can never alternate sets (each reload costs ~2.7us and
    # this kernel interleaves Exp/Ln tightly). Ids must stay aligned with
    # act_info.json, so keep every set name but empty the others.
    if not getattr(bacc, "_act_tables_pinned", False):
        _orig_gat = bacc.get_activation_tables

        def _pinned_tables(arch):
            t = _orig_gat(arch)
            keep = "natural_log_exp_and_others"
            return {k: (v if k == keep else set()) for k, v in t.items()}

        bacc.get_activation_tables = _pinned_tables
        bacc._act_tables_pinned = True

    f32 = mybir.dt.float32
    bf16 = mybir.dt.float16  # fp16: 2-byte like bf16 but 8x tighter mantissa
    AF = mybir.ActivationFunctionType
    ALU = mybir.AluOpType

    nc = bacc.Bacc(None)

    # ---- DRAM parameters (per core) ----
    qT_d = nc.declare_dram_parameter("qT", [DM, S], f32, isOutput=False)
    kT_d = nc.declare_dram_parameter("kT", [DM, S], f32, isOutput=False)
    vT_d = nc.declare_dram_parameter("vT", [DM, S], f32, isOutput=False)
    WqT_d = nc.declare_dram_parameter("WqT", [DM, DM], f32, isOutput=False)
    WkT_d = nc.declare_dram_parameter("WkT", [DM, DM], f32, isOutput=False)
    WvT_d = nc.declare_dram_parameter("WvT", [DM, DM], f32, isOutput=False)
    WoT_d = nc.declare_dram_parameter("WoT", [DM, DM], f32, isOutput=False)
    bq_d = nc.declare_dram_parameter("bq2d", [DM, 1], f32, isOutput=False)
    bk_d = nc.declare_dram_parameter("bk2d", [DM, 1], f32, isOutput=False)
    bv_d = nc.declare_dram_parameter("bvrow", [1, DM], f32, isOutput=False)
    bo_d = nc.declare_dram_parameter("bo2d", [DM, 1], f32, isOutput=False)
    lngam_d = nc.declare_dram_parameter("lngam_bc", [128, H], f32, isOutput=False)
    lnpos_d = nc.declare_dram_parameter("lnpos", [QB, 128, S], f32, isOutput=False)
    negSU_d = nc.declare_dram_parameter("negSU", [128, 128], f32, isOutput=False)
    I128_d = nc.declare_dram_parameter("I128", [128, 128], f32, isOutput=False)
    yT_d = nc.declare_dram_parameter("yT", [DM, S], f32, isOutput=True)

    NDMC = DM // 128  # 4 chunks of the model dim

    with tile.TileContext(nc) as tc:
        import contextlib

        persist = tc.tile_pool(name="persist", bufs=1)
        with contextlib.ExitStack() as stack:
            pp = stack.enter_context(persist)

            qhT = [pp.tile([128, S], f32, tag=f"qhT{i}", name=f"qhT{i}") for i in range(NDMC)]
            khT = [pp.tile([128, S], f32, tag=f"khT{i}", name=f"khT{i}") for i in range(NDMC)]
            vh = [pp.tile([128, DM], bf16, tag=f"vh{i}", name=f"vh{i}") for i in range(QB)]
            ccT = [pp.tile([128, S], f32, tag=f"ccT{t}", name=f"ccTs{t}") for t in range(NDMC)]
            I128 = pp.tile([128, 128], f32, tag="I128", name="I128")
            negSU = pp.tile([128, 128], f32, tag="negSU", name="negSU")
            I128b = pp.tile([128, 128], bf16, tag="I128b", name="I128b")
            lngam = pp.tile([128, H], f32, tag="lngam", name="lngam")
            bq = [pp.tile([128, 1], f32, tag=f"bq{t}", name=f"bq{t}") for t in range(NDMC)]
            bk = [pp.tile([128, 1], f32, tag=f"bk{t}", name=f"bk{t}") for t in range(NDMC)]
            bo = [pp.tile([128, 1], f32, tag=f"bo{t}", name=f"bo{t}") for t in range(NDMC)]

            bump = pp.tile([128, 1], f32, tag="bump", name="bump")
            nc.vector.memset(bump, 1.0 + 2.0 ** -22)
            nc.sync.dma_start(out=I128, in_=I128_d[:, :])
            nc.sync.dma_start(out=negSU, in_=negSU_d[:, :])
            nc.sync.dma_start(out=lngam, in_=lngam_d[:, :])
            nc.vector.tensor_copy(I128b, I128)
            for t in range(NDMC):
                nc.sync.dma_start(out=bq[t], in_=bq_d[t * 128:(t + 1) * 128, :])
                nc.sync.dma_start(out=bk[t], in_=bk_d[t * 128:(t + 1) * 128, :])
                nc.sync.dma_start(out=bo[t], in_=bo_d[t * 128:(t + 1) * 128, :])

            # ---------- phase 1: projections ----------
            with tc.tile_pool(name="p1in", bufs=1) as p1in, \
                 tc.tile_pool(name="p1ps", bufs=3, space="PSUM") as p1ps:
                qTt = [p1in.tile([128, S], f32, tag=f"qT{t}", name=f"qTs{t}") for t in range(NDMC)]
                kTt = [p1in.tile([128, S], f32, tag=f"kT{t}", name=f"kTs{t}") for t in range(NDMC)]
                vTt = [p1in.tile([128, S], f32, tag=f"vT{t}", name=f"vTs{t}") for t in range(NDMC)]
                Wq = [p1in.tile([128, DM], f32, tag=f"Wq{t}", name=f"Wqs{t}") for t in range(NDMC)]
                Wk = [p1in.tile([128, DM], f32, tag=f"Wk{t}", name=f"Wks{t}") for t in range(NDMC)]
                Wv = [p1in.tile([128, DM], f32, tag=f"Wv{t}", name=f"Wvs{t}") for t in range(NDMC)]
                ones1 = p1in.tile([1, 128], f32, tag="ones1", name="ones1")
                bvr = p1in.tile([1, DM], f32, tag="bvr", name="bvr")
                nc.vector.memset(ones1, 1.0)
                nc.sync.dma_start(out=bvr, in_=bv_d[:, :])
                for t in range(NDMC):
                    sl = slice(t * 128, (t + 1) * 128)
                    nc.sync.dma_start(out=qTt[t], in_=qT_d[sl, :])
                    nc.sync.dma_start(out=kTt[t], in_=kT_d[sl, :])
                    nc.sync.dma_start(out=vTt[t], in_=vT_d[sl, :])
                    nc.sync.dma_start(out=Wq[t], in_=WqT_d[sl, :])
                    nc.sync.dma_start(out=Wk[t], in_=WkT_d[sl, :])
                    nc.sync.dma_start(out=Wv[t], in_=WvT_d[sl, :])

                # qhT/khT: [d', s] = sum_dm WxT[dm, d'] * xT[dm, s]  (+ bias)
                for dst, W, xT, bias in ((qhT, Wq, qTt, bq), (khT, Wk, kTt, bk)):
                    for i in range(NDMC):           # d' block
                        for n in range(S // 512):   # s chunk
                            ps = p1ps.tile([128, 512], f32, tag="pp", name="pp")
                            for t in range(NDMC):   # dm chunk
                                nc.tensor.matmul(
                                    ps,
                                    lhsT=W[t][:, i * 128:(i + 1) * 128],
                                    rhs=xT[t][:, n * 512:(n + 1) * 512],
                                    start=(t == 0), stop=(t == NDMC - 1),
                                )
                            nc.vector.tensor_scalar_add(
                                dst[i][:, n * 512:(n + 1) * 512], ps, bias[i],
                            )
                # vh: [s, d'] = sum_dm vT[dm, s-block]^T * WvT[dm, d'] (+ bv)
                for sb in range(QB):
                    ps = p1ps.tile([128, DM], f32, tag="pp", name="pp")
                    for t in range(NDMC):
                        nc.tensor.matmul(
                            ps,
                            lhsT=vTt[t][:, sb * 128:(sb + 1) * 128],
                            rhs=Wv[t][:, :],
                            start=(t == 0), stop=False,
                        )
                    nc.tensor.matmul(ps, lhsT=ones1, rhs=bvr, start=False, stop=True)
                    nc.vector.tensor_copy(vh[sb], ps)  # fp32 psum -> bf16

            # ---------- phase 2: attention (h outer, qb inner) ----------
            with tc.tile_pool(name="p2w", bufs=4) as p2w, \
                 tc.tile_pool(name="p2t", bufs=8) as p2t, \
                 tc.tile_pool(name="lnp", bufs=3) as lnp, \
                 tc.tile_pool(name="att", bufs=2) as attp, \
                 tc.tile_pool(name="ps_s", bufs=2, space="PSUM") as ps_s, \
                 tc.tile_pool(name="ps_t", bufs=2, space="PSUM") as ps_t, \
                 tc.tile_pool(name="ps_o", bufs=1, space="PSUM") as ps_o:
                for h in range(H):
                    hi, hr = h // 2, (h % 2) * 64
                    # attn^T strips for this head: block (kb, qb) lives at
                    # [:, kb*S + qb*128 : kb*S + (qb+1)*128]
                    attnT = attp.tile([128, QB * S], bf16, tag="attnT", name="attnT")
                    attnT3 = attnT.rearrange("p (kb q) -> p kb q", kb=QB)
                    for qb in range(QB):
                        W = (qb + 1) * 128
                        lnpos = lnp.tile([128, S], f32, tag="lnpos", name="lnpos")
                        nc.sync.dma_start(out=lnpos[:, :W], in_=lnpos_d[qb, :, :W])
                        qa = qhT[hi][hr:hr + 64, qb * 128:(qb + 1) * 128]
                        pss = ps_s.tile([128, 1024], f32, tag="pss", name="pss")
                        bounds = [(c, min(c + 512, W)) for c in range(0, W, 512)]
                        for (c0, c1) in bounds:
                            is_diag = c1 == W
                            nc.tensor.matmul(
                                pss[:, c0:c1],
                                lhsT=qa, rhs=khT[hi][hr:hr + 64, c0:c1],
                                start=True, stop=not is_diag,
                            )
                        nc.tensor.matmul(
                            pss[:, qb * 128:W], lhsT=I128, rhs=negSU,
                            start=False, stop=True,
                        )
                        # two parallel PSUM readers: exp on ACT, raw copy on DVE
                        sraw = p2w.tile([128, S], f32, tag="sraw", name="sraw")
                        nc.vector.tensor_copy(sraw[:, :W], pss[:, :W])
                        u = p2w.tile([128, S], f32, tag="u", name="u")
                        nc.scalar.activation(u[:, :W], pss[:, :W], AF.Exp, scale=0.125)
                        cs = p2w.tile([128, S], f32, tag="cs", name="cs")
                        nc.vector.tensor_tensor_scan(
                            cs[:, :W], data0=u[:, :W], data1=u[:, :W],
                            initial=0.0, op0=ALU.add, op1=ALU.bypass,
                        )
                        nrden = p2t.tile([128, 1], f32, tag="nrden", name="nrden")
                        nc.vector.reciprocal(nrden, cs[:, W - 1:W])
                        nc.vector.tensor_scalar_mul(nrden, nrden, -1.0)
                        # lnr = ln(R/denom + ~2^-22); suffix mass ratio in log domain
                        lnr = p2w.tile([128, S], f32, tag="lnr", name="lnr")
                        nc.scalar.activation(
                            lnr[:, :W], cs[:, :W], AF.Ln, scale=nrden,
                            bias=bump,
                        )
                        # lnsum = lnr + lnpos, in place (frees a tile slot)
                        nc.gpsimd.tensor_tensor(
                            lnr[:, :W], lnr[:, :W], lnpos[:, :W], op=ALU.add,
                        )
                        tg = p2w.tile([128, S], f32, tag="tg", name="tg")
                        nc.scalar.activation(
                            tg[:, :W], lnr[:, :W], AF.Exp, scale=0.5,
                            bias=lngam[:, h:h + 1],
                        )
                        nc.scalar.activation(tg[:, :W], tg[:, :W], AF.Exp, scale=-1.0)
                        nc.vector.tensor_tensor(
                            tg[:, :W], sraw[:, :W], tg[:, :W], op=ALU.mult,
                        )
                        denom2 = p2t.tile([128, 1], f32, tag="den2", name="den2")
                        nc.scalar.activation(
                            tg[:, :W], tg[:, :W], AF.Exp, scale=0.125,
                            accum_out=denom2,
                        )
                        rden2 = p2t.tile([128, 1], f32, tag="rden2", name="rden2")
                        nc.vector.reciprocal(rden2, denom2)
                        attn = p2w.tile([128, S], bf16, tag="attn", name="attn")
                        nc.vector.tensor_scalar_mul(attn[:, :W], tg[:, :W], rden2)
                        # transpose attn blocks into per-kb strips (fp16, PE)
                        pst = ps_t.tile([128, 1024], bf16, tag="pst", name="pst")
                        for kb in range(qb + 1):
                            c0 = kb * 128
                            nc.tensor.matmul(
                                pst[:, c0:c0 + 128], lhsT=attn[:, c0:c0 + 128],
                                rhs=I128b, is_transpose=True,
                                start=(kb == 0), stop=(kb == qb),
                            )
                        nc.vector.tensor_copy(
                            attnT3[:, 0:qb + 1, qb * 128:(qb + 1) * 128],
                            pst.rearrange("p (kb q) -> p kb q", kb=QB)[:, 0:qb + 1, :],
                        )
                    # attn@V for this head: out^T[d, q] += vh_kb^T @ attnT_kb
                    pso = ps_o.tile([64, S], f32, tag="pso", name="pso")
                    for bank in range(2):           # psum bank = 512 q columns
                        q0 = bank * 512
                        kbs = [kb for kb in range(QB) if kb * 128 < q0 + 512]
                        for n, kb in enumerate(kbs):
                            c0 = max(kb * 128, q0)
                            nc.tensor.matmul(
                                pso[:, c0:q0 + 512],
                                lhsT=vh[kb][:, h * DK:(h + 1) * DK],
                                rhs=attnT3[:, kb, c0:q0 + 512],
                                start=(n == 0), stop=(n == len(kbs) - 1),
                            )
                    nc.vector.tensor_copy(ccT[hi][hr:hr + 64, :], pso)

            # ---------- phase 3: output projection ----------
            with tc.tile_pool(name="p3w", bufs=1) as p3w, \
                 tc.tile_pool(name="ps_y", bufs=2, space="PSUM") as ps_y:
                Wo = [p3w.tile([128, DM], f32, tag=f"Wo{t}", name=f"Wos{t}") for t in range(NDMC)]
                for t in range(NDMC):
                    nc.sync.dma_start(out=Wo[t], in_=WoT_d[t * 128:(t + 1) * 128, :])
                yTt = [p3w.tile([128, S], f32, tag=f"yT{m}", name=f"yTt{m}") for m in range(NDMC)]
                for mb in range(NDMC):
                    for n in range(S // 512):
                        psy = ps_y.tile([128, 512], f32, tag="psy", name="psy")
                        for t in range(NDMC):
                            nc.tensor.matmul(
                                psy,
                                lhsT=Wo[t][:, mb * 128:(mb + 1) * 128],
                                rhs=ccT[t][:, n * 512:(n + 1) * 512],
                                start=(t == 0), stop=(t == NDMC - 1),
                            )
                        nc.vector.tensor_scalar_add(
                            yTt[mb][:, n * 512:(n + 1) * 512], psy, bo[mb],
                        )
                for mb in range(NDMC):
                    nc.sync.dma_start(
                        out=yT_d[mb * 128:(mb + 1) * 128, :], in_=yTt[mb][:, :],
                    )

    nc.compile()
    return nc


def _get_program():
    if "nc" not in _CACHE:
        _CACHE["nc"] = _build_program()
    return _CACHE["nc"]


def _host_prep(inputs):
    """Build the per-core input maps from the full problem inputs."""
    f = np.float32
    q, k, v = inputs["q"], inputs["k"], inputs["v"]
    gammas = np.asarray(inputs["gammas"], dtype=f).reshape(H)

    common = {
        "WqT": np.ascontiguousarray(np.asarray(inputs["Wq"], f).T),
        "WkT": np.ascontiguousarray(np.asarray(inputs["Wk"], f).T),
        "WvT": np.ascontiguousarray(np.asarray(inputs["Wv"], f).T),
        "WoT": np.ascontiguousarray(np.asarray(inputs["Wo"], f).T),
        "bq2d": np.asarray(inputs["bq"], f).reshape(DM, 1).copy(),
        "bk2d": np.asarray(inputs["bk"], f).reshape(DM, 1).copy(),
        "bvrow": np.asarray(inputs["bv"], f).reshape(1, DM).copy(),
        "bo2d": np.asarray(inputs["bo"], f).reshape(DM, 1).copy(),
    }
    # ln(softplus(gamma_h)), broadcast to all 128 partitions
    lngam = np.log(np.logaddexp(0.0, gammas.astype(np.float64))).astype(f)
    common["lngam_bc"] = np.ascontiguousarray(np.broadcast_to(lngam, (128, H)))
    # ln|q-j| strips: strip[qb][i, j] = ln(qb*128 + i - j) for valid, else -1e4
    i_idx = np.arange(128)[:, None]
    j_idx = np.arange(S)[None, :]
    lnpos = np.full((QB, 128, S), -1e4, dtype=f)
    for qb in range(QB):
        pos = qb * 128 + i_idx - j_idx
        with np.errstate(divide="ignore", invalid="ignore"):
            lp = np.where(pos > 0, np.log(np.maximum(pos, 1)), -1e4)
        lnpos[qb] = lp.astype(f)
    common["lnpos"] = lnpos
    common["I128"] = np.eye(128, dtype=f)
    iu = np.triu(np.ones((128, 128), dtype=bool), k=1)
    common["negSU"] = np.where(iu, f(-1e30), f(0.0)).astype(f)

    in_maps = []
    for c in range(NCORES):
        m = dict(common)
        m["qT"] = np.ascontiguousarray(np.asarray(q[c], f).T)
        m["kT"] = np.ascontiguousarray(np.asarray(k[c], f).T)
        m["vT"] = np.ascontiguousarray(np.asarray(v[c], f).T)
        in_maps.append(m)
    return in_maps


def run(inputs, trace=False):
    """Run on the 8 NeuronCores; returns (output [B,S,DM], BassKernelResults)."""
    from concourse.bass_utils import run_bass_kernel_spmd

    nc = _get_program()
    in_maps = _host_prep(inputs)
    res = run_bass_kernel_spmd(nc, in_maps, list(range(NCORES)), trace=trace)
    out = np.stack(
        [np.ascontiguousarray(res.results[c]["yT"].T) for c in range(NCORES)]
    ).astype(np.float32)
    return out, res


def kernel(**inputs) -> np.ndarray:
    out, _ = run(inputs)
    return out
